# revision 17
# baseline (speedup 1.0000x reference)
"""Trainium2 Bass kernel for DecoderRNNWithAttention (teacher-forced LSTM decoder).

Key mathematical simplification: the attention block is an exact no-op.
The encoder output has a single spatial position, so softmax over that
axis is exactly 1.0 and context == features, independent of h. Hence:
  - the enc/dec/full attention projections never affect the output;
  - the input-side gate contributions Gx = X @ W_ih.T + (b_ih + b_hh)
    can be precomputed for all T steps in one batched matmul
    (X_t = [word_t ; features]);
  - the serial recurrence is only gates_t = Gx_t + h_t @ W_hh.T plus the
    LSTM elementwise cell; logits_t = h_{t+1} @ fcn_W.T + fcn_b.

Sharding: phases 1-2 data-parallel over batch (8 cores x 16 rows).
Phase 3 (the vocab projection, ~50% of FLOPs-time) is tensor-parallel
over V: the bf16 h history is AllGathered in 3 time-chunks (each AG
overlaps the remaining recurrence steps), then each core computes a
4000-row vocab slice of the logits for ALL 128 batch rows with its
fcn weight slice fully resident in SBUF (65MB -> 8.4MB of weight DMA
per core, matmul N=512).

Recurrence step structure (the trace showed 5.9us/step of idle PE after
each 256-matmul gate burst, waiting on a ~10-op serial elementwise
chain): the burst is split i,f -> g -> o with separate PSUM tiles, so
the adds/sigmoid/tanh/cell-update for i,f,g run while the PE is still
streaming the g/o matmuls. Only add_o -> sigmoid(o) -> h=sig_o*tanh(c)
remains after the burst (~1.4us).

Device layouts (all "transposed" so the partition dim is the feature dim):
  - gate dim 4H split into 32 slices of 128, permuted [i f o g]: cols of
    the per-step gate tile are (slice j)*16 + batch.
  - h state history hall[128, t*128 + k*16 + b] (k = H-tile), written
    once per step as one [128, 128] tile; the recurrence matmul rhs.
  - h history for fcn: hexp[128, q*1024 + k*128 + tq*16 + b] bf16
    (q = t-chunk of 8 steps), DMA'd per chunk to DRAM as [k*128+p, 128],
    AllGather-concatenated over ranks, fetched back as per-chunk tiles
    hfq[q][k][128, r*128 + tq*16 + b].
  - vocab projection: out tiles [vocab-slice-tile 128, 3072 rows],
    rows = (q, r, tq, b).
"""

import numpy as np
import ml_dtypes

import concourse.bacc as bacc
import concourse.mybir as mybir
import concourse.tile as tile
from concourse.bass_utils import run_bass_kernel_spmd

B, T, E, H, V, ENC = 128, 25, 512, 1024, 32000, 400
NCORES = 8
BS = B // NCORES          # 16 batch rows per core
TB = T * BS               # 400 = matmul N for phase 1
KT = H // 128             # 8 K-tiles
GS = 4 * H // 128         # 32 gate slices
XDIM = E + ENC            # 912, padded to 1024
VS = V // NCORES          # 4000 vocab rows per core (phase 3 V-shard)
VSP = 4096                # padded to whole 128-tiles
VT3 = VSP // 128          # 32 vocab tiles per core
NQ = 3                    # t-chunks for the h AllGather
QS = (T - 1) // NQ        # 8 steps per chunk
QC = QS * B               # 1024 phase-3 row-cols per chunk (r, tq, b)
ROWS3 = NQ * QC           # 3072

# torch LSTMCell gate order is [i f g o]; we want [i f o g] so one
# sigmoid covers i,f and o sits in its own group (computed last).
# perm_src[j] = source slice for permuted block j.
PERM_SRC = list(range(0, 16)) + list(range(24, 32)) + list(range(16, 24))

CFG = {"p1": "bf16", "rec": "bf16", "fcn": "bf16", "gx": "bf16"}

_F32 = mybir.dt.float32
_BF16 = mybir.dt.bfloat16
_DT = {"f32": mybir.dt.float32, "bf16": mybir.dt.bfloat16}
_NPDT = {"f32": np.float32, "bf16": ml_dtypes.bfloat16}


def build_nc(cfg=CFG):
    AF = mybir.ActivationFunctionType
    p1, rec, fcn, gx = cfg["p1"], cfg["rec"], cfg["fcn"], cfg["gx"]

    nc = bacc.Bacc(num_devices=NCORES)
    xT_d = nc.dram_tensor("xT", [128, KT * TB], _DT[p1], kind="ExternalInput")
    wih_d = nc.dram_tensor("wih", [128, KT * 4 * H], _DT[p1], kind="ExternalInput")
    whh_d = nc.dram_tensor("whh", [128, KT * 4 * H], _DT[rec], kind="ExternalInput")
    fcnw_d = nc.dram_tensor("fcnw", [128, KT * VSP], _DT[fcn], kind="ExternalInput")
    bsum_d = nc.dram_tensor("bsum", [128, GS], _F32, kind="ExternalInput")
    fb_d = nc.dram_tensor("fb", [128, VT3], _F32, kind="ExternalInput")
    out_d = nc.dram_tensor("out", [VT3, 128, ROWS3], _F32, kind="ExternalOutput")

    with tile.TileContext(nc) as tc:
        with (
            tc.tile_pool(name="pers", bufs=1) as pers,
            tc.tile_pool(name="psum", bufs=4, space="PSUM") as psum,
            tc.tile_pool(name="elem", bufs=2) as elem,
            tc.tile_pool(name="dram", bufs=1, space="DRAM") as dram,
        ):
            hall = pers.tile([128, T * 128], _DT[rec])
            hexp = pers.tile([128, NQ * KT * 128], _BF16)  # (q, k, tq, b)
            bsum_sb = pers.tile([128, GS], _F32)
            fb_sb = pers.tile([128, VT3], _F32)
            fcnw_sb = pers.tile([128, KT * VSP], _DT[fcn])
            # released after phase 1 / phase 2 respectively (LIFO order)
            xtp = tc.alloc_tile_pool(name="xtp", bufs=1)
            whhp = tc.alloc_tile_pool(name="whhp", bufs=1)
            gxtp = tc.alloc_tile_pool(name="gxtp", bufs=1)
            xt_sb = xtp.tile([128, KT * TB], _DT[p1], name="xt_sb")
            whh_sb = whhp.tile([128, KT * 4 * H], _DT[rec], name="whh_sb")
            gxt = gxtp.tile([128, GS * TB], _DT[gx], name="gxt")

            hexp_d = [dram.tile([KT * 128, 128], _BF16, name=f"hexp_d{q}")
                      for q in range(NQ)]
            hag_d = [dram.tile([NCORES * KT * 128, 128], _BF16,
                               name=f"hag_d{q}", addr_space="Shared")
                     for q in range(NQ)]

            nc.sync.dma_start(xt_sb[:], xT_d[:])
            nc.sync.dma_start(bsum_sb[:], bsum_d[:])
            nc.sync.dma_start(fb_sb[:], fb_d[:])
            nc.gpsimd.memset(hall[:], 0.0)

            # ---------------- Phase 1: Gx = X @ W_ih.T + (b_ih + b_hh) ----
            wih_v = wih_d.rearrange("p (k c) -> p k c", k=KT)
            with nc.named_scope("p1"), tc.tile_pool(name="wihp", bufs=2) as wihp:
                for chunk in range(8):
                    wih_sb = wihp.tile([128, KT * 512], _DT[p1], tag="wih")
                    nc.sync.dma_start(
                        wih_sb.rearrange("p (k c) -> p k c", k=KT)[:],
                        wih_v[:, :, chunk * 512:(chunk + 1) * 512])
                    for jj in range(4):
                        j = chunk * 4 + jj
                        ps = psum.tile([128, TB], _F32, tag="ps", name="ps", bufs=3)
                        for k in range(KT):
                            nc.tensor.matmul(
                                ps[:],
                                wih_sb[:, k * 512 + jj * 128:
                                       k * 512 + jj * 128 + 128],
                                xt_sb[:, k * TB:(k + 1) * TB],
                                start=(k == 0), stop=(k == KT - 1))
                        nc.scalar.activation(
                            gxt[:, j * TB:(j + 1) * TB], ps[:], AF.Identity,
                            bias=bsum_sb[:, j:j + 1])

            # W_hh + fcn weights load on gpsimd: their multi-us DMA
            # issues would interleave with phase-1's wih chunk loads on
            # the sync queue and starve the phase-1 matmuls.
            for k in range(KT):
                nc.gpsimd.dma_start(whh_sb[:, k * 4096:(k + 1) * 4096],
                                    whh_d[:, k * 4096:(k + 1) * 4096])
            for k in range(KT):
                nc.gpsimd.dma_start(fcnw_sb[:, k * VSP:(k + 1) * VSP],
                                    fcnw_d[:, k * VSP:(k + 1) * VSP])

            # ---------------- Phase 2: LSTM recurrence --------------------
            # gxt viewed as [128, slice j, t, b]
            gxt_r = gxt.rearrange("p (j t b) -> p j (t b)", j=GS, t=T, b=BS)
            # permuted gate groups: j 0:16 = i,f | 16:24 = o | 24:32 = g
            hexp_r = hexp.rearrange("p (q k tq b) -> p (q k) (tq b)",
                                    q=NQ, k=KT, tq=QS, b=BS)

            def mm_group(ps_t, j0, j1, t):
                for j in range(j0, j1):
                    for k in range(KT):
                        nc.tensor.matmul(
                            ps_t[:, (j - j0) * BS:(j - j0) * BS + BS],
                            whh_sb[:, k * 4096 + j * 128:
                                   k * 4096 + j * 128 + 128],
                            hall[:, (t - 1) * 128 + k * BS:
                                 (t - 1) * 128 + k * BS + BS],
                            start=(k == 0), stop=(k == KT - 1))

            with nc.named_scope("p2"):
                c_prev = None
                for t in range(T):
                    ts = slice(t * BS, (t + 1) * BS)
                    if t == 0:
                        sig_if = elem.tile([128, 256], _F32, tag="sif", name="sif")
                        nc.scalar.activation(
                            sig_if.rearrange("p (a b) -> p a b", b=BS)[:],
                            gxt_r[:, 0:16, ts], AF.Sigmoid)
                        tg = elem.tile([128, 128], _F32, tag="tg", name="tg")
                        nc.scalar.activation(
                            tg.rearrange("p (a b) -> p a b", b=BS)[:],
                            gxt_r[:, 24:32, ts], AF.Tanh)
                        cn = elem.tile([128, 128], _F32, tag="c", name="cn")
                        nc.vector.tensor_mul(cn[:], sig_if[:, 0:128], tg[:])
                        thc = elem.tile([128, 128], _F32, tag="thc", name="thc")
                        nc.scalar.activation(thc[:], cn[:], AF.Tanh)
                        sig_o = elem.tile([128, 128], _F32, tag="so", name="so")
                        nc.scalar.activation(
                            sig_o.rearrange("p (a b) -> p a b", b=BS)[:],
                            gxt_r[:, 16:24, ts], AF.Sigmoid)
                    else:
                        ps_if = psum.tile([128, 256], _F32, tag="pif",
                                          name="pif", bufs=1)
                        ps_g = psum.tile([128, 128], _F32, tag="pg",
                                         name="pg", bufs=1)
                        ps_o = psum.tile([128, 128], _F32, tag="po",
                                         name="po", bufs=1)
                        mm_group(ps_if, 0, 16, t)
                        mm_group(ps_g, 24, 32, t)
                        mm_group(ps_o, 16, 24, t)
                        # i,f path: runs while the PE streams the g/o groups
                        g_if = elem.tile([128, 256], _F32, tag="gif", name="gif")
                        nc.vector.tensor_add(
                            g_if.rearrange("p (a b) -> p a b", b=BS)[:],
                            ps_if.rearrange("p (a b) -> p a b", b=BS)[:],
                            gxt_r[:, 0:16, ts])
                        sig_if = elem.tile([128, 256], _F32, tag="sif", name="sif")
                        nc.scalar.activation(sig_if[:], g_if[:], AF.Sigmoid)
                        # g path
                        g_g = elem.tile([128, 128], _F32, tag="gg", name="gg")
                        nc.vector.tensor_add(
                            g_g.rearrange("p (a b) -> p a b", b=BS)[:],
                            ps_g.rearrange("p (a b) -> p a b", b=BS)[:],
                            gxt_r[:, 24:32, ts])
                        tg = elem.tile([128, 128], _F32, tag="tg", name="tg")
                        nc.scalar.activation(tg[:], g_g[:], AF.Tanh)
                        # cell update on GpSimd (SBUF-only ops): the DVE
                        # FIFO holds the PSUM-reading adds, and its
                        # add_o (which must wait for the full burst)
                        # would otherwise head-of-line block these
                        # earlier-ready ops.
                        cn = elem.tile([128, 128], _F32, tag="c", name="cn")
                        nc.gpsimd.tensor_mul(cn[:], sig_if[:, 128:256], c_prev[:])
                        t1 = elem.tile([128, 128], _F32, tag="t1", name="t1")
                        nc.gpsimd.tensor_mul(t1[:], sig_if[:, 0:128], tg[:])
                        nc.gpsimd.tensor_add(cn[:], cn[:], t1[:])
                        thc = elem.tile([128, 128], _F32, tag="thc", name="thc")
                        nc.scalar.activation(thc[:], cn[:], AF.Tanh)
                        # o path: the only post-burst work
                        g_o = elem.tile([128, 128], _F32, tag="go", name="go")
                        nc.vector.tensor_add(
                            g_o.rearrange("p (a b) -> p a b", b=BS)[:],
                            ps_o.rearrange("p (a b) -> p a b", b=BS)[:],
                            gxt_r[:, 16:24, ts])
                        sig_o = elem.tile([128, 128], _F32, tag="so", name="so")
                        nc.scalar.activation(sig_o[:], g_o[:], AF.Sigmoid)
                    nc.vector.tensor_mul(hall[:, t * 128:(t + 1) * 128],
                                         sig_o[:], thc[:])
                    if t > 0:
                        # bf16 (q,k)-major copy for the fcn phase
                        q, tq = (t - 1) // QS, (t - 1) % QS
                        hsrc = hall.rearrange("p (t k b) -> p t k b",
                                              t=T, k=KT, b=BS)
                        nc.vector.tensor_copy(
                            hexp_r[:, q * KT:(q + 1) * KT,
                                   tq * BS:(tq + 1) * BS],
                            hsrc[:, t, :, :])
                        if tq == QS - 1:
                            # chunk complete: ship it + AllGather
                            nc.sync.dma_start(
                                hexp_d[q].rearrange("(k r) c -> r k c",
                                                    k=KT)[:],
                                hexp.rearrange("p (q k c) -> p q k c",
                                               q=NQ, k=KT)[:, q, :, :])
                            nc.gpsimd.collective_compute(
                                "AllGather",
                                mybir.AluOpType.bypass,
                                replica_groups=[list(range(NCORES))],
                                ins=[hexp_d[q].opt()],
                                outs=[hag_d[q].opt()],
                            )
                    c_prev = cn

            # Phase 2 scratch is dead; phase 3's h tiles reuse the space.
            gxtp.release()
            whhp.release()
            xtp.release()

            hfp = tc.alloc_tile_pool(name="hfp", bufs=1)
            hfq = []
            for q in range(NQ):
                # [rank, k, p, c] view of this chunk's gathered h history
                hag_v = hag_d[q].rearrange("(r k p) c -> k p r c",
                                           r=NCORES, k=KT)
                hfk = []
                for k in range(KT):
                    t_ = hfp.tile([128, B * QS], _BF16, name=f"hf{q}_{k}")
                    nc.gpsimd.dma_start(
                        t_.rearrange("p (r c) -> p r c", r=NCORES)[:],
                        hag_v[k])
                    hfk.append(t_)
                hfq.append(hfk)

            # ---------------- Phase 3: logits = H @ fcn_W.T + fcn_b -------
            # V-sharded: this core's 4096-padded vocab slice, all 3072 rows.
            with nc.named_scope("p3"), tc.tile_pool(name="outp", bufs=4) as outp:
                for q in range(NQ):
                    for vt in range(VT3):
                        ot = outp.tile([128, QC], _F32, tag="ot", name="ot")
                        # both 512-row halves accumulate under the same
                        # stationary tile: halves the LDWEIGHTS count so
                        # the weight path fully hides under the stream
                        ps0 = psum.tile([128, 512], _F32, tag="ps",
                                        name="psf0", bufs=3)
                        ps1 = psum.tile([128, 512], _F32, tag="ps1",
                                        name="psf1", bufs=2)
                        for k in range(KT):
                            w = fcnw_sb[:, k * VSP + vt * 128:
                                        k * VSP + vt * 128 + 128]
                            nc.tensor.matmul(ps0[:], w, hfq[q][k][:, 0:512],
                                             start=(k == 0), stop=(k == KT - 1))
                            nc.tensor.matmul(ps1[:], w, hfq[q][k][:, 512:1024],
                                             start=(k == 0), stop=(k == KT - 1))
                        nc.scalar.activation(ot[:, 0:512], ps0[:], AF.Identity,
                                             bias=fb_sb[:, vt:vt + 1])
                        nc.scalar.activation(ot[:, 512:1024], ps1[:],
                                             AF.Identity,
                                             bias=fb_sb[:, vt:vt + 1])
                        # alternate the issuing engine: DMA-issue costs
                        # ~0.7us each and would serialize on one queue
                        eng = nc.gpsimd if vt % 2 else nc.sync
                        eng.dma_start(
                            out_d[vt][:, q * QC:(q + 1) * QC], ot[:])
            hfp.release()

    nc.finalize()
    return nc


def _prep_shared(W_ih, W_hh, b_ih, b_hh, cfg):
    """Host-side layout transforms (no FLOPs beyond the bias sum)."""
    perm = np.concatenate([np.arange(s * 128, (s + 1) * 128) for s in PERM_SRC])
    p1np, recnp = _NPDT[cfg["p1"]], _NPDT[cfg["rec"]]

    wihT = np.zeros((H, 4 * H), np.float32)
    wihT[:XDIM, :] = np.asarray(W_ih, np.float32)[perm].T
    wih_t = np.ascontiguousarray(
        wihT.reshape(KT, 128, 4 * H).transpose(1, 0, 2).reshape(128, KT * 4 * H)
    ).astype(p1np)

    whhT = np.asarray(W_hh, np.float32)[perm].T  # [H, 4H]
    whh_t = np.ascontiguousarray(
        whhT.reshape(KT, 128, 4 * H).transpose(1, 0, 2).reshape(128, KT * 4 * H)
    ).astype(recnp)

    bsum = (np.asarray(b_ih, np.float32) + np.asarray(b_hh, np.float32))[perm]
    bsum_t = np.ascontiguousarray(bsum.reshape(GS, 128).T)
    return {"wih": wih_t, "whh": whh_t, "bsum": bsum_t}


def _prep_core(features, captions, emb_W, fcn_W, fcn_b, core, cfg):
    p1np, fcnnp = _NPDT[cfg["p1"]], _NPDT[cfg["fcn"]]
    sl = slice(core * BS, (core + 1) * BS)
    feats = np.asarray(features, np.float32)[sl]          # [16, ENC]
    caps = np.asarray(captions)[sl]                       # [16, T]
    embW = np.asarray(emb_W, np.float32)

    words = np.empty((BS, T, E), np.float32)
    words[:, 0, :] = embW[1]
    words[:, 1:, :] = embW[caps[:, :-1]]

    xpad = np.zeros((H, TB), np.float32)                  # [1024, 400]
    xpad[:E] = words.transpose(2, 1, 0).reshape(E, TB)    # (e, t, b)
    xpad[E:XDIM] = np.broadcast_to(
        feats.T[:, None, :], (ENC, T, BS)).reshape(ENC, TB)
    xT_t = np.ascontiguousarray(
        xpad.reshape(KT, 128, TB).transpose(1, 0, 2).reshape(128, KT * TB)
    ).astype(p1np)

    # this core's vocab slice of the fcn projection, padded to 4096
    fw = np.zeros((VSP, H), np.float32)
    fw[:VS] = np.asarray(fcn_W, np.float32)[core * VS:(core + 1) * VS]
    fcnw_t = np.ascontiguousarray(
        fw.T.reshape(KT, 128, VSP).transpose(1, 0, 2).reshape(128, KT * VSP)
    ).astype(fcnnp)
    fbp = np.zeros(VSP, np.float32)
    fbp[:VS] = np.asarray(fcn_b, np.float32)[core * VS:(core + 1) * VS]
    fb_t = np.ascontiguousarray(fbp.reshape(VT3, 128).T)
    return {"xT": xT_t, "fcnw": fcnw_t, "fb": fb_t}


_BUILT = {}


def kernel(features, captions, emb_W, W_ih, W_hh, b_ih, b_hh,
           enc_W, enc_b, dec_W, dec_b, full_W, full_b, fcn_W, fcn_b,
           _cfg=None, _trace=False):
    cfg = dict(CFG if _cfg is None else _cfg)
    key = tuple(sorted(cfg.items()))
    if key not in _BUILT:
        _BUILT[key] = build_nc(cfg)
    nc = _BUILT[key]

    shared = _prep_shared(W_ih, W_hh, b_ih, b_hh, cfg)
    in_maps = []
    for c in range(NCORES):
        m = dict(shared)
        m.update(_prep_core(features, captions, emb_W, fcn_W, fcn_b, c, cfg))
        in_maps.append(m)

    res = run_bass_kernel_spmd(nc, in_maps, list(range(NCORES)), trace=_trace)

    out = np.empty((B, T - 1, V), np.float32)
    for c in range(NCORES):
        o = np.asarray(res.results[c]["out"])             # [32, 128, 3072]
        o = o.reshape(VSP, NQ, NCORES, QS, BS)            # (v, q, r, tq, b)
        # batch = r*16+b, t = q*8+tq
        o = o.transpose(2, 4, 1, 3, 0).reshape(B, T - 1, VSP)
        out[:, :, c * VS:(c + 1) * VS] = o[:, :, :VS]
    kernel._last_result = res
    return out
